# revision 4
# baseline (speedup 1.0000x reference)
"""KPRN (knowledge-path recurrent network) Trainium2 kernel.

Data-parallel over the batch-pair dim B=1024 across 8 NeuronCores
(128 pairs / core).  Full (unsharded) inputs in, full outputs back.

Per-core dataflow (all activations kept feature-major, i.e. transposed,
so the PE contraction dim sits on SBUF partitions):

  doc rows   --indirect DMA-->  [128,768] --PE transpose--> docT
             --matmul+ELU+matmul+tanh--> newsT [128E, 256]  (item1|item2)
  entity rows --indirect DMA--> [128,100] --PE transpose--> entT
             --matmul+tanh--> node_embT [128E, 7680]
  relation branch folded on host:  relX = tanh(rel_table@rc_W+rc_b) @ Wr.T
    augmented with a ones-row one-hot so each gate matmul picks up
    rel contribution AND the LSTM bias in a single K=61 matmul.
  LSTM: per 512-seq block, per step: gates accumulate in one 4-bank PSUM
    tile [128,2048] (slots i|f|o|g) from node/rel/h matmuls; one fused
    sigmoid ACTIVATE over [i|f|o], tanh for g, DVE cell update.
  MLP + logsumexp/sigmoid/log epilogue on-device; per-core loss partial
  summed on host (exact: each core owns exactly 128 of the 1024 terms).
"""

import sys

sys.path.insert(0, "/opt/trn_rl_repo")

import numpy as np

import concourse.bass as bass
import concourse.tile as tile
from concourse import mybir
from concourse.bass_utils import run_bass_kernel_spmd
from concourse.masks import make_identity

F32 = mybir.dt.float32
I32 = mybir.dt.int32
AF = mybir.ActivationFunctionType

M = 8                      # cores
B, P, L = 1024, 20, 3
T = L + 2                  # 5
E = 128
D_DOC, D_ENT = 768, 100
N_DOC, N_ENT, N_REL = 50000, 200000, 60
GAMMA = 2.0
EPS = 1e-7

BC = B // M                # 128 pairs per core
N = BC * P                 # 2560 sequences per core
NB = 512                   # LSTM block width (one PSUM bank)
NBLK = N // NB             # 5
NCH = (N * L) // 128       # 60 entity gather chunks of 128 rows


def _declare_io(nc):
    d = {}

    def inp(name, shape, dt=F32):
        d[name] = nc.dram_tensor(name, list(shape), dt, kind="ExternalInput")

    def outp(name, shape, dt=F32):
        d[name] = nc.dram_tensor(name, list(shape), dt, kind="ExternalOutput")

    inp("doc_table", (N_DOC, D_DOC))
    inp("entity_table", (N_ENT, D_ENT))
    inp("item_idx", (BC, 2), I32)
    inp("paths_idx", (128, NCH), I32)
    inp("oh", (N_REL + 1, N * T))
    inp("ncW1", (128, D_DOC))          # col-chunk k = nc_W1[128k:128k+128, :]
    inp("ncW2", (E, E))
    inp("ncb1", (E, 1))
    inp("ncb1n", (E, 1))
    inp("ncb2a", (E, 1))
    inp("ecW", (D_ENT, E))
    inp("ecb", (E, 1))
    inp("wnT", (E, 4 * E))
    inp("whhT", (E, 4 * E))
    inp("relX", (N_REL + 1, 4 * E))    # row 60 = b_ih + b_hh
    inp("mlpW1", (E, E))
    inp("mlpb1", (E, 1))
    inp("mlpW2", (E, 1))
    inp("mlpb2", (1, 1))
    inp("label", (BC, 1))
    outp("scores_out", (N,))
    outp("pred_out", (BC,))
    outp("loss_out", (1,))
    return d


def _emit(ctx, tc, io):
    nc = tc.nc
    ap = {k: v.ap() for k, v in io.items()}

    const = ctx.enter_context(tc.tile_pool(name="const", bufs=1))
    persist = ctx.enter_context(tc.tile_pool(name="persist", bufs=1))

    # ---- constants / weights into SBUF ----
    ident = const.tile([128, 128], F32)
    make_identity(nc, ident[:])
    ones_col = const.tile([128, 1], F32)
    nc.gpsimd.memset(ones_col[:], 1.0)

    def load(name, shape, dt=F32):
        t = const.tile(list(shape), dt, name=f"w_{name}")
        nc.sync.dma_start(t[:], ap[name][:])
        return t

    ncW1 = load("ncW1", (128, D_DOC))
    ncW2 = load("ncW2", (E, E))
    ncb1 = load("ncb1", (E, 1))
    ncb1n = load("ncb1n", (E, 1))
    ncb2a = load("ncb2a", (E, 1))
    ecW = load("ecW", (D_ENT, E))
    ecb = load("ecb", (E, 1))
    wnT = load("wnT", (E, 4 * E))
    whhT = load("whhT", (E, 4 * E))
    relX = load("relX", (N_REL + 1, 4 * E))
    mlpW1 = load("mlpW1", (E, E))
    mlpb1 = load("mlpb1", (E, 1))
    mlpW2 = load("mlpW2", (E, 1))
    mlpb2 = load("mlpb2", (1, 1))
    label = load("label", (BC, 1))
    item_idx = load("item_idx", (BC, 2), I32)
    paths_idx = load("paths_idx", (128, NCH), I32)

    # ---- persistent activations ----
    newsx1 = persist.tile([128, N], F32)     # news1 per sequence (expanded x20)
    newsx2 = persist.tile([128, N], F32)
    node_embT = persist.tile([128, N * L], F32)
    hlast = persist.tile([128, N], F32)

    # ================= doc gather + news compress =================
    with (
        tc.tile_pool(name="docrows", bufs=2) as drp,
        tc.tile_pool(name="doctp", bufs=4, space="PSUM") as dtp,
        tc.tile_pool(name="docsb", bufs=2) as dsb,
        tc.tile_pool(name="docps", bufs=2, space="PSUM") as dps,
    ):
        docT = dsb.tile([128, 2 * D_DOC], F32, name="docT")
        for c in range(2):
            rows = drp.tile([128, D_DOC], F32, name="docrow")
            nc.gpsimd.indirect_dma_start(
                out=rows[:],
                out_offset=None,
                in_=ap["doc_table"][:],
                in_offset=bass.IndirectOffsetOnAxis(ap=item_idx[:, c : c + 1], axis=0),
            )
            for k in range(6):
                pt = dtp.tile([128, 128], F32, name="doct_ps")
                nc.tensor.transpose(pt[:], rows[:, 128 * k : 128 * (k + 1)], ident[:])
                nc.vector.tensor_copy(
                    docT[:, 256 * k + 128 * c : 256 * k + 128 * (c + 1)], pt[:]
                )
        h1p = dps.tile([128, 256], F32, name="h1ps")
        for k in range(6):
            nc.tensor.matmul(
                h1p[:],
                lhsT=ncW1[:, 128 * k : 128 * (k + 1)],
                rhs=docT[:, 256 * k : 256 * (k + 1)],
                start=(k == 0),
                stop=(k == 5),
            )
        # ELU(x) = relu(x) + exp(-relu(-x)) - 1 ; the -1 is folded into ncb2a
        r = dsb.tile([128, 256], F32, name="elu_r")
        nc.scalar.activation(r[:], h1p[:], AF.Relu, bias=ncb1[:, 0:1])
        rn = dsb.tile([128, 256], F32, name="elu_rn")
        nc.scalar.activation(rn[:], h1p[:], AF.Relu, bias=ncb1n[:, 0:1], scale=-1.0)
        ex = dsb.tile([128, 256], F32, name="elu_e")
        nc.scalar.activation(ex[:], rn[:], AF.Exp, scale=-1.0)
        h1a = dsb.tile([128, 256], F32, name="elu_sum")
        nc.vector.tensor_add(h1a[:], r[:], ex[:])
        np2 = dps.tile([128, 256], F32, name="newsps")
        nc.tensor.matmul(np2[:], lhsT=ncW2[:], rhs=h1a[:], start=True, stop=True)
        newsT = dsb.tile([128, 256], F32, name="newsT")
        nc.scalar.activation(newsT[:], np2[:], AF.Tanh, bias=ncb2a[:, 0:1])
        # expand per-pair -> per-sequence (x20)
        x1v = newsx1.rearrange("p (n r) -> p n r", r=P)
        x2v = newsx2.rearrange("p (n r) -> p n r", r=P)
        for j in range(P):
            nc.vector.tensor_copy(x1v[:, :, j], newsT[:, 0:128])
            nc.vector.tensor_copy(x2v[:, :, j], newsT[:, 128:256])

    # ================= entity gather + compress =================
    with (
        tc.tile_pool(name="entrows", bufs=6) as erp,
        tc.tile_pool(name="enttp", bufs=6, space="PSUM") as etp,
        tc.tile_pool(name="entg", bufs=3) as egp,
        tc.tile_pool(name="entps", bufs=2, space="PSUM") as eps_,
    ):
        for g in range(NCH // 4):          # 15 groups of 4 chunks
            ent_g = egp.tile([D_ENT, 512], F32, name="ent_g")
            for q in range(4):
                c = 4 * g + q
                rows = erp.tile([128, D_ENT], F32, name="entrow")
                nc.gpsimd.indirect_dma_start(
                    out=rows[:],
                    out_offset=None,
                    in_=ap["entity_table"][:],
                    in_offset=bass.IndirectOffsetOnAxis(
                        ap=paths_idx[:, c : c + 1], axis=0
                    ),
                )
                pt = etp.tile([128, 128], F32, name="entt_ps")
                nc.tensor.transpose(pt[:D_ENT, :], rows[:], ident[:])
                nc.vector.tensor_copy(ent_g[:, 128 * q : 128 * (q + 1)], pt[:D_ENT, :])
            cp = eps_.tile([128, 512], F32, name="entcps")
            nc.tensor.matmul(cp[:], lhsT=ecW[:], rhs=ent_g[:], start=True, stop=True)
            nc.scalar.activation(
                node_embT[:, 512 * g : 512 * (g + 1)], cp[:], AF.Tanh, bias=ecb[:, 0:1]
            )

    # ================= LSTM =================
    node3 = node_embT.rearrange("p (n l) -> p n l", l=L)
    SLOT = (0, 1, 3, 2)  # torch gate order i,f,g,o -> psum slots [i|f|o|g]
    with (
        tc.tile_pool(name="ohp", bufs=2) as ohp,
        tc.tile_pool(name="gpsum", bufs=2, space="PSUM") as gps,
        tc.tile_pool(name="ifop", bufs=2) as ifop,
        tc.tile_pool(name="ggp", bufs=2) as ggp,
        tc.tile_pool(name="cp", bufs=3) as cpp,
        tc.tile_pool(name="tmp", bufs=4) as tmpp,
        tc.tile_pool(name="hp", bufs=3) as hpp,
    ):
        for blk in range(NBLK):
            nb = slice(NB * blk, NB * (blk + 1))
            ohb = ohp.tile([N_REL + 1, NB * T], F32, name="ohb")
            nc.sync.dma_start(ohb[:], ap["oh"][:, NB * T * blk : NB * T * (blk + 1)])
            ohv = ohb.rearrange("k (n t) -> k n t", t=T)
            h_prev = None
            c_prev = None
            for t in range(T):
                gp = gps.tile([128, 2048], F32, name="gpsum")
                if t == 0:
                    node_rhs = newsx1[:, nb]
                elif t == T - 1:
                    node_rhs = newsx2[:, nb]
                else:
                    node_rhs = node3[:, nb, t - 1]
                oh_rhs = ohv[:, :, t]
                for m in range(4):
                    s = SLOT[m]
                    out_ap = gp[:, 512 * s : 512 * (s + 1)]
                    nc.tensor.matmul(
                        out_ap,
                        lhsT=wnT[:, 128 * m : 128 * (m + 1)],
                        rhs=node_rhs,
                        start=True,
                        stop=False,
                    )
                    nc.tensor.matmul(
                        out_ap,
                        lhsT=relX[:, 128 * m : 128 * (m + 1)],
                        rhs=oh_rhs,
                        start=False,
                        stop=(t == 0),
                    )
                    if t > 0:
                        nc.tensor.matmul(
                            out_ap,
                            lhsT=whhT[:, 128 * m : 128 * (m + 1)],
                            rhs=h_prev[:],
                            start=False,
                            stop=True,
                        )
                ifo = ifop.tile([128, 1536], F32, name="ifo")
                nc.scalar.activation(ifo[:], gp[:, 0:1536], AF.Sigmoid)
                gg = ggp.tile([128, 512], F32, name="gg")
                nc.scalar.activation(gg[:], gp[:, 1536:2048], AF.Tanh)
                c_cur = cpp.tile([128, NB], F32, name="c")
                if t == 0:
                    nc.vector.tensor_mul(c_cur[:], ifo[:, 0:512], gg[:])
                else:
                    t_ig = tmpp.tile([128, NB], F32, name="t_ig")
                    nc.vector.tensor_mul(t_ig[:], ifo[:, 0:512], gg[:])
                    t_fc = tmpp.tile([128, NB], F32, name="t_fc")
                    nc.vector.tensor_mul(t_fc[:], ifo[:, 512:1024], c_prev[:])
                    nc.vector.tensor_add(c_cur[:], t_ig[:], t_fc[:])
                tcc = tmpp.tile([128, NB], F32, name="tanh_c")
                nc.scalar.activation(tcc[:], c_cur[:], AF.Tanh)
                if t < T - 1:
                    h_cur = hpp.tile([128, NB], F32, name="h")
                    nc.gpsimd.tensor_mul(h_cur[:], ifo[:, 1024:1536], tcc[:])
                    h_prev = h_cur
                else:
                    nc.gpsimd.tensor_mul(hlast[:, nb], ifo[:, 1024:1536], tcc[:])
                c_prev = c_cur

    # ================= MLP + scores =================
    with (
        tc.tile_pool(name="mps", bufs=2, space="PSUM") as mps,
        tc.tile_pool(name="sps", bufs=1, space="PSUM") as sps,
        tc.tile_pool(name="m1p", bufs=2) as m1p,
        tc.tile_pool(name="scp", bufs=1) as scp_pool,
    ):
        sp = sps.tile([128, N], F32, name="spsum")  # only partition 0 used
        for q in range(NBLK):
            mp = mps.tile([128, 512], F32, name="mlpps")
            nc.tensor.matmul(
                mp[:], lhsT=mlpW1[:], rhs=hlast[:, 512 * q : 512 * (q + 1)],
                start=True, stop=True,
            )
            m1 = m1p.tile([128, 512], F32, name="m1")
            nc.scalar.activation(m1[:], mp[:], AF.Relu, bias=mlpb1[:, 0:1])
            nc.tensor.matmul(
                sp[:1, 512 * q : 512 * (q + 1)], lhsT=mlpW2[:, 0:1], rhs=m1[:],
                start=True, stop=True,
            )
        scoresT = scp_pool.tile([1, N], F32, name="scoresT")
        nc.scalar.activation(scoresT[:1, :], sp[:1, :], AF.Identity, bias=mlpb2[:1, 0:1])
        nc.sync.dma_start(ap["scores_out"].rearrange("(a n) -> a n", a=1), scoresT[:1, :])

    # ================= logsumexp / sigmoid / loss =================
    with (
        tc.tile_pool(name="dramp", bufs=1, space="DRAM") as dramp,
        tc.tile_pool(name="epi", bufs=1) as epi,
        tc.tile_pool(name="epips", bufs=1, space="PSUM") as epips,
    ):
        sc_dram = dramp.tile([1, N], F32, name="sc_rt")
        nc.sync.dma_start(sc_dram[:], scoresT[:1, :])
        scp = epi.tile([BC, P], F32, name="scores_pair")
        nc.sync.dma_start(scp[:], sc_dram.rearrange("a (p q) -> (a p) q", q=P)[:])

        m0 = epi.tile([BC, 1], F32, name="m0")
        nc.vector.reduce_max(m0[:], scp[:], axis=mybir.AxisListType.X)
        mh = epi.tile([BC, 1], F32, name="mh")          # -max/GAMMA
        nc.scalar.mul(mh[:], m0[:], -1.0 / GAMMA)
        ex = epi.tile([BC, P], F32, name="expx")
        nc.scalar.activation(ex[:], scp[:], AF.Exp, bias=mh[:, 0:1], scale=1.0 / GAMMA)
        ssum = epi.tile([BC, 1], F32, name="ssum")
        nc.vector.reduce_sum(ssum[:], ex[:], axis=mybir.AxisListType.X)
        lse = epi.tile([BC, 1], F32, name="lse")
        nc.scalar.activation(lse[:], ssum[:], AF.Ln)
        lsef = epi.tile([BC, 1], F32, name="lsef")      # ln(sum) + max/GAMMA
        nc.vector.tensor_sub(lsef[:], lse[:], mh[:])
        en = epi.tile([BC, 1], F32, name="en")
        nc.scalar.activation(en[:], lsef[:], AF.Exp, scale=-1.0)
        u = epi.tile([BC, 1], F32, name="u")
        nc.vector.tensor_scalar_add(u[:], en[:], 1.0)
        praw = epi.tile([BC, 1], F32, name="praw")
        nc.vector.reciprocal(praw[:], u[:])
        nc.sync.dma_start(ap["pred_out"].rearrange("(p a) -> p a", a=1), praw[:])
        pc = epi.tile([BC, 1], F32, name="pclip")
        nc.vector.tensor_scalar(
            pc[:], praw[:], scalar1=EPS, scalar2=1.0 - EPS,
            op0=mybir.AluOpType.max, op1=mybir.AluOpType.min,
        )
        lp = epi.tile([BC, 1], F32, name="logp")
        nc.scalar.activation(lp[:], pc[:], AF.Ln)
        omp = epi.tile([BC, 1], F32, name="om_p")
        nc.vector.tensor_scalar(
            omp[:], pc[:], scalar1=-1.0, scalar2=1.0,
            op0=mybir.AluOpType.mult, op1=mybir.AluOpType.add,
        )
        l1p = epi.tile([BC, 1], F32, name="log1mp")
        nc.scalar.activation(l1p[:], omp[:], AF.Ln)
        dd = epi.tile([BC, 1], F32, name="dd")
        nc.vector.tensor_sub(dd[:], lp[:], l1p[:])
        t1 = epi.tile([BC, 1], F32, name="t1")
        nc.vector.tensor_mul(t1[:], label[:], dd[:])
        t2 = epi.tile([BC, 1], F32, name="t2")
        nc.vector.tensor_add(t2[:], t1[:], l1p[:])
        lsum = epips.tile([1, 1], F32, name="losspsum")
        nc.tensor.matmul(lsum[:1, :1], lhsT=t2[:], rhs=ones_col[:], start=True, stop=True)
        lout = epi.tile([1, 1], F32, name="lossout")
        nc.scalar.copy(lout[:1, :], lsum[:1, :])
        nc.sync.dma_start(ap["loss_out"].rearrange("(a b) -> a b", a=1), lout[:1, :])


_NC_CACHE = {}


def _get_nc():
    if "nc" not in _NC_CACHE:
        from concourse import bacc

        nc = bacc.Bacc("TRN2", target_bir_lowering=False, debug=False, num_devices=M)
        io = _declare_io(nc)
        from contextlib import ExitStack

        with tile.TileContext(nc) as tc:
            with ExitStack() as ctx:
                _emit(ctx, tc, io)
        nc.compile()
        _NC_CACHE["nc"] = nc
    return _NC_CACHE["nc"]


def _prep_in_maps(inputs):
    f32 = lambda k: np.asarray(inputs[k], dtype=np.float32)
    idx = lambda k: np.asarray(inputs[k]).astype(np.int32)

    item1, item2 = idx("item1"), idx("item2")
    paths, edges = idx("paths"), idx("edges")
    label = f32("label")
    doc_table = np.ascontiguousarray(f32("doc_table"))
    entity_table = np.ascontiguousarray(f32("entity_table"))
    rel_table = f32("relation_table")
    nc_W1, nc_b1 = f32("nc_W1"), f32("nc_b1")
    nc_W2, nc_b2 = f32("nc_W2"), f32("nc_b2")
    ec_W, ec_b = f32("ec_W"), f32("ec_b")
    rc_W, rc_b = f32("rc_W"), f32("rc_b")
    W_ih, W_hh = f32("W_ih"), f32("W_hh")
    b_ih, b_hh = f32("b_ih"), f32("b_hh")
    mlp_W1, mlp_b1 = f32("mlp_W1"), f32("mlp_b1")
    mlp_W2, mlp_b2 = f32("mlp_W2"), f32("mlp_b2")

    # ---- shared (weight-only) preprocessing ----
    relc = np.tanh(rel_table @ rc_W + rc_b)                 # [60, E]
    Wr = W_ih[:, E : 2 * E]                                 # [4E, E]
    relX = relc @ Wr.T                                      # [60, 4E]
    bias = (b_ih + b_hh)[None, :]                           # [1, 4E]
    relXa = np.ascontiguousarray(np.vstack([relX, bias]).astype(np.float32))

    shared = {
        "doc_table": doc_table,
        "entity_table": entity_table,
        "ncW1": np.ascontiguousarray(
            nc_W1.reshape(6, 128, 128).transpose(1, 0, 2).reshape(128, 6 * 128)
        ),
        "ncW2": np.ascontiguousarray(nc_W2),
        "ncb1": np.ascontiguousarray(nc_b1.reshape(E, 1)),
        "ncb1n": np.ascontiguousarray(-nc_b1.reshape(E, 1)),
        "ncb2a": np.ascontiguousarray((nc_b2 - nc_W2.sum(axis=0)).reshape(E, 1)),
        "ecW": np.ascontiguousarray(ec_W),
        "ecb": np.ascontiguousarray(ec_b.reshape(E, 1)),
        "wnT": np.ascontiguousarray(W_ih[:, :E].T),
        "whhT": np.ascontiguousarray(W_hh.T),
        "relX": relXa,
        "mlpW1": np.ascontiguousarray(mlp_W1),
        "mlpb1": np.ascontiguousarray(mlp_b1.reshape(E, 1)),
        "mlpW2": np.ascontiguousarray(mlp_W2),
        "mlpb2": np.ascontiguousarray(mlp_b2.reshape(1, 1)),
    }

    in_maps = []
    jj = np.arange(N)
    for c in range(M):
        sl = slice(BC * c, BC * (c + 1))
        e_c = edges[sl].reshape(N, L)
        oh = np.zeros((N_REL + 1, N * T), np.float32)
        oh[N_REL, :] = 1.0                                  # bias row
        for t in range(L):
            oh[e_c[:, t], jj * T + t] = 1.0
        oh[0, jj * T + L] = 1.0
        oh[0, jj * T + L + 1] = 1.0
        pf = paths[sl].reshape(-1)                          # [N*L]
        m = dict(shared)
        m.update(
            {
                "item_idx": np.ascontiguousarray(
                    np.stack([item1[sl], item2[sl]], axis=1).astype(np.int32)
                ),
                "paths_idx": np.ascontiguousarray(pf.reshape(NCH, 128).T),
                "oh": oh,
                "label": np.ascontiguousarray(label[sl].reshape(BC, 1)),
            }
        )
        in_maps.append(m)
    return in_maps


def kernel(**inputs):
    in_maps = _prep_in_maps(inputs)
    nc = _get_nc()
    res = run_bass_kernel_spmd(nc, in_maps, list(range(M)))
    scores = np.concatenate(
        [res.results[c]["scores_out"] for c in range(M)]
    ).reshape(B, P)
    predicts = np.concatenate([res.results[c]["pred_out"] for c in range(M)])
    total = np.float32(sum(float(res.results[c]["loss_out"][0]) for c in range(M)))
    loss = np.float32(-total / B)
    return loss, predicts, scores
